# revision 68
# baseline (speedup 1.0000x reference)
"""Multi-head attention (B=2, T=S=2048, E=1024, H=16, D=64) on 8 NeuronCores.

Sharding: core = (batch, head-group-of-4).  Each core computes the full
attention for 4 heads of one batch plus that slice's out-projection
contribution; the host sums the 4 partials per batch.

Device math (per core, matmuls bf16, accumulation fp32 in PSUM):
  qT = (scale*Wq_c) @ x_q     -> [256, T]   (jd on partitions)
  kT = Wk_c @ x_k             -> [256, SP]
  v  = x_v @ Wv_c^T           -> [SP, 256]  (s on partitions), plus ones col
  scoresT[s,t] = kT^T.qT      (K=64 per head; head pair shares a psum tile)
  pw = exp(scoresT) * ebm     (ebm = exp(bias^T)*keep, host-precomputed;
                               no max-subtraction: scores are O(10))
  pvU[t, d|den] = pw_chunk^T @ [v|1]   (flipped PV: 65-col matmuls, the
                               ones column gives the softmax denominator)
  pv_bf = pvU[:, 0:64] * (1/den)       (normalize + bf16 convert, DVE)
  pvT   = transpose(pv_bf)             (PE transpose via identity)
  out   = pvT^T @ Wo_c^T               -> [T, E] f16 partials

The four 65-col PV accumulation slots share one PSUM bank: the first
matmul into the bank carries start=True, which marks the whole 2KB
zero-region pending, so each slot's first write lazily zeroes itself.
"""

import numpy as np
import ml_dtypes

B, T, S, E = 2, 2048, 2048, 1024
H, D = 16, 64
SCALING = float(D) ** -0.5
HEADS = 4              # heads per core
JD = HEADS * D         # 256 projected dims per core
NCORES = 8

BF16 = ml_dtypes.bfloat16


# scheduling knobs (A/B-swept via TimelineSim; see test harness)
TUNE = {
    "defer_dve": 2,     # PV emission lag behind its sc step (DVE-mul steps)
    "defer_pool": 5,    # same for Pool-mul steps
    "pool_si": (2, 4),  # steps whose exp*ebm mul runs on Pool
    "flush_si": 0,      # step at which the previous block's tail PVs emit
    "tail_split": 0,
    "dma_tp": 0,
    "fp8_scores": 0,    # q/k in fp8e4 + DoubleRow scores matmuls (fails precision)
    "pool_si_drain": (0, 2, 4),  # pool_si override for drain-carrying blocks        # transpose pv via DMA xbar instead of PE+DVE    # split tail out-copies across DVE+ACT, per-ei DMAs
}


def _build_nc(SP=S):
    """SP = padded count of unmasked (kept) keys, multiple of 128.

    Masked keys are compacted away on the host: K/V/ebm arrive with only
    the kept keys (zero-padded to SP).  Padding rows have ebm == 0, so
    their probability is exactly 0.
    """
    import concourse.bass as bass
    import concourse.mybir as mybir
    import concourse.tile as tile
    from concourse import bacc
    from concourse.masks import make_identity
    from contextlib import ExitStack

    DT = mybir.dt.bfloat16
    F8 = mybir.dt.float8e4
    F32 = mybir.dt.float32
    F16 = mybir.dt.float16
    Act = mybir.ActivationFunctionType

    EC = E // 128        # 8 contraction chunks for projections
    MC = JD // 128       # 2 partition-chunks of the per-core head dims
    SC = SP // 128       # kept-key chunks
    SCH = (SC + 1) // 2  # first-half kept-key chunks (x-load pipelining)
    NJB = T // 512       # 4 t-blocks of 512

    sc_half = [list(range(0, SCH)), list(range(SCH, SC))]

    nc = bacc.Bacc("TRN2", target_bir_lowering=False, debug=False)

    xqT = nc.dram_tensor("xqT", [E, T], DT, kind="ExternalInput")
    xkT = nc.dram_tensor("xkT", [E, SP], DT, kind="ExternalInput")
    xvT = nc.dram_tensor("xvT", [E, SP], DT, kind="ExternalInput")
    ebm4 = nc.dram_tensor("ebm4", [NJB, SP, 512], DT, kind="ExternalInput")
    wqT = nc.dram_tensor("wqT", [E, JD], DT, kind="ExternalInput")
    wkT = nc.dram_tensor("wkT", [E, JD], DT, kind="ExternalInput")
    wvT = nc.dram_tensor("wvT", [E, JD], DT, kind="ExternalInput")
    woT = nc.dram_tensor("woT", [JD, E], DT, kind="ExternalInput")
    bqk = nc.dram_tensor("bqk", [128, 2 * MC], F32, kind="ExternalInput")
    out_p = nc.dram_tensor("out_p", [T, E], F16, kind="ExternalOutput")

    xqr = xqT.rearrange("(c p) t -> p c t", p=128)
    xkr = xkT.rearrange("(c p) s -> p c s", p=128)
    xvr = xvT.rearrange("(c p) s -> p c s", p=128)
    er = ebm4.rearrange("j (c p) t -> j p c t", p=128)

    with tile.TileContext(nc) as tc, ExitStack() as ctx:
        const = ctx.enter_context(tc.tile_pool(name="const", bufs=1))
        persist = ctx.enter_context(tc.tile_pool(name="persist", bufs=1))

        ident = const.tile([128, 128], DT)
        make_identity(nc, ident)

        bqk_sb = const.tile([128, 2 * MC], F32)
        bq_sb = bqk_sb[:, 0:MC]
        bk_sb = bqk_sb[:, MC : 2 * MC]
        wq_sb = const.tile([128, EC, JD], DT)
        wk_sb = const.tile([128, EC, JD], DT)
        wv_sb = const.tile([128, EC, JD], DT)
        wo_sb = const.tile([128, MC, E], DT)

        QKDT = F8 if TUNE["fp8_scores"] else DT
        if TUNE["fp8_scores"]:
            # second k-tile plane is zeros: DoubleRow contracts 2 planes of
            # K=64; plane 1 contributes nothing but doubles the stream rate
            qT_sb = persist.tile([128, 2, MC, T], QKDT)
            kT_sb = persist.tile([128, 2, MC, SP], QKDT)
            nc.gpsimd.memset(qT_sb[:, 1, :, :], 0.0)
            nc.gpsimd.memset(kT_sb[:, 1, :, :], 0.0)
        else:
            qT_sb = persist.tile([128, 1, MC, T], QKDT)
            kT_sb = persist.tile([128, 1, MC, SP], QKDT)
        vone_sb = persist.tile([128, SC, HEADS, D + 1], DT)
        ebm_sb = persist.tile([128, SC, NJB, 512], DT)
        nc.vector.memset(vone_sb[:, :, :, D : D + 1], 1.0)

        with (
            tc.tile_pool(name="xq", bufs=4) as xqp,
            tc.tile_pool(name="xkv", bufs=4) as xkvp,
            tc.tile_pool(name="flow", bufs=6) as flow,
            tc.tile_pool(name="pvbf", bufs=6) as pvbfp,
            tc.tile_pool(name="small", bufs=4) as small,
            tc.tile_pool(name="pvt", bufs=2) as pvtp,
            tc.tile_pool(name="ot", bufs=3) as otp,
            tc.tile_pool(name="scps", bufs=2, space="PSUM") as sc_ps,
            tc.tile_pool(name="pvps", bufs=2, space="PSUM") as pv_ps,
        ):
            # ---- x input tiles (column-chunked for pipelining) ----
            xq_t = [
                xqp.tile([128, EC, 512], DT, tag="xq", name=f"xq{q}")
                for q in range(4)
            ]
            xk_t, xv_t = [], []
            for half in range(2):
                w = len(sc_half[half]) * 128
                xk_t.append(xkvp.tile([128, EC, w], DT, tag="xkv", name=f"xk{half}"))
                xv_t.append(xkvp.tile([128, EC, w], DT, tag="xkv", name=f"xv{half}"))

            def dma_x(dst, src, c0, c1):
                nc.sync.dma_start(out=dst[:], in_=src[:, :, c0:c1])

            def dma_ebm(jb, half):
                scr = sc_half[half]
                nc.sync.dma_start(
                    out=ebm_sb[:, scr[0] : scr[-1] + 1, jb, :],
                    in_=er[jb, :, scr[0] : scr[-1] + 1, :],
                )

            # ---- DMA stream (order = transfer order, feeds block 1 first) ----
            nc.sync.dma_start(out=wk_sb[:], in_=wkT.rearrange("(c p) j -> p c j", p=128))
            nc.sync.dma_start(out=wq_sb[:], in_=wqT.rearrange("(c p) j -> p c j", p=128))
            nc.sync.dma_start(out=bqk_sb[:], in_=bqk[:, :])
            xh0w = SCH * 128
            nc.sync.dma_start(out=xk_t[0][:, :, 0 : xh0w // 2],
                              in_=xkr[:, :, 0 : xh0w // 2])
            nc.sync.dma_start(out=xk_t[0][:, :, xh0w // 2 : xh0w],
                              in_=xkr[:, :, xh0w // 2 : xh0w])
            dma_x(xq_t[0], xqr, 0, 512)
            dma_ebm(0, 0)
            nc.sync.dma_start(out=wv_sb[:], in_=wvT.rearrange("(c p) j -> p c j", p=128))
            dma_x(xv_t[0], xvr, 0, SCH * 128)
            dma_x(xk_t[1], xkr, SCH * 128, SP)
            dma_ebm(0, 1)
            dma_x(xv_t[1], xvr, SCH * 128, SP)
            dma_x(xq_t[1], xqr, 512, 1024)
            dma_ebm(1, 0)
            dma_ebm(1, 1)

            # ---- PE keep-alive bridge: the p-state model degrades matmul
            # throughput after any PE idle stretch longer than ~3us, so pace
            # tiny matmuls off a DVE delay chain until the first real work.
            delay_t = []
            for i in range(3):
                dtile = const.tile([1, 2400], DT)
                nc.vector.memset(dtile[:], 1.0)
                delay_t.append(dtile)

            # ---- projection works (finely split for emission scheduling) ----
            def k_work(half, mc, pool, tag="proj", ostep=512, act=False):
                scr = sc_half[half]
                c0 = scr[0] * 128
                w = len(scr) * 128
                for o0 in range(0, w, ostep):
                    ow = min(ostep, w - o0)
                    ps = pool.tile([128, 512], F32, tag=tag)
                    for ec in range(EC):
                        nc.tensor.matmul(
                            ps[:, 0:ow],
                            lhsT=wk_sb[:, ec, mc * 128 : (mc + 1) * 128],
                            rhs=xk_t[half][:, ec, o0 : o0 + ow],
                            start=(ec == 0),
                            stop=(ec == EC - 1),
                        )
                    if act:
                        nc.scalar.activation(
                            kT_sb[:, 0, mc, c0 + o0 : c0 + o0 + ow],
                            ps[:, 0:ow], Act.Identity,
                            bias=bqk_sb[:, MC + mc : MC + mc + 1])
                    else:
                        nc.vector.tensor_scalar_add(
                            kT_sb[:, 0, mc, c0 + o0 : c0 + o0 + ow],
                            ps[:, 0:ow],
                            bqk_sb[:, MC + mc : MC + mc + 1],
                        )

            def v_work(sc, hp, pool, tag="proj"):
                half = 0 if sc < SCH else 1
                lc = sc - sc_half[half][0]
                ps = pool.tile([128, 512], F32, tag=tag)
                for ec in range(EC):
                    nc.tensor.matmul(
                        ps[:, 0:128],
                        lhsT=xv_t[half][:, ec, lc * 128 : (lc + 1) * 128],
                        rhs=wv_sb[:, ec, hp * 128 : (hp + 1) * 128],
                        start=(ec == 0),
                        stop=(ec == EC - 1),
                    )
                nc.vector.tensor_copy(
                    vone_sb[:, sc, 2 * hp : 2 * hp + 2, 0:D],
                    ps[:, 0:128].rearrange("p (h d) -> p h d", h=2),
                )

            def q_work(mc, q, proj_ps, tag="proj", act=False):
                ps = proj_ps.tile([128, 512], F32, tag=tag)
                for ec in range(EC):
                    nc.tensor.matmul(
                        ps[:],
                        lhsT=wq_sb[:, ec, mc * 128 : (mc + 1) * 128],
                        rhs=xq_t[q][:, ec, :],
                        start=(ec == 0),
                        stop=(ec == EC - 1),
                    )
                if act:
                    nc.scalar.activation(
                        qT_sb[:, 0, mc, q * 512 : (q + 1) * 512],
                        ps[:], Act.Identity, bias=bqk_sb[:, mc : mc + 1])
                else:
                    nc.vector.tensor_scalar_add(
                        qT_sb[:, 0, mc, q * 512 : (q + 1) * 512],
                        ps[:],
                        bqk_sb[:, mc : mc + 1],
                    )

            # ---- attention block: (th, hp, j) = SC sc-steps + normalize ----
            # The PE stream is software-pipelined: PV matmuls for step si are
            # emitted 1 step later (2 for Pool-multiplied steps), so the
            # in-order PE queue never waits on the exp->mul chain.  The last
            # PV + normalize are deferred into the next block via `flush`.
            def attn_block(th, hp, j, flush_prev=None, works_at=None,
                           pv_defer=None, pool_si=None):
                if pv_defer is None:
                    pv_defer = TUNE["defer_dve"]
                if pool_si is None:
                    pool_si = TUNE["pool_si"]
                jb = th * 2 + j
                t0 = jb * 512
                pvu = [
                    pv_ps.tile([128, 4, D + 1], F32, tag="pv", name=f"pv{jb}_{hp}_{hh}")
                    for hh in range(2)
                ]
                pend = []   # (si, pw, due)
                works_at = works_at or {}
                first_emit = [True]

                def emit_pv(si, pw):
                    last = si == SC - 1
                    for hh in range(2):
                        h = 2 * hp + hh
                        for tcc in range(4):
                            # start only on each bank's first-emitted matmul:
                            # it marks the whole 2KB zero-region pending, so
                            # each slot's first write lazily zeroes itself.
                            # (deferred emission can reorder si, so key the
                            # start off emission order, not si)
                            nc.tensor.matmul(
                                pvu[hh][:, tcc, :],
                                lhsT=pw[:, hh * 512 + tcc * 128 : hh * 512 + (tcc + 1) * 128],
                                rhs=vone_sb[:, si, h, :],
                                start=(first_emit[0] and tcc == 0),
                                stop=(last and tcc == 3),
                                skip_group_check=True,
                            )
                    first_emit[0] = False

                for si in range(SC):
                    ps = sc_ps.tile([128, 1024], F32, tag="sc")
                    for hh in range(2):
                        par = hh * 64
                        if TUNE["fp8_scores"]:
                            nc.tensor.matmul(
                                ps[:, hh * 512 : (hh + 1) * 512],
                                lhsT=kT_sb[par : par + 64, :, hp,
                                           si * 128 : (si + 1) * 128],
                                rhs=qT_sb[par : par + 64, :, hp, t0 : t0 + 512],
                                start=True,
                                stop=True,
                                perf_mode=mybir.MatmulPerfMode.DoubleRow,
                            )
                        else:
                            nc.tensor.matmul(
                                ps[:, hh * 512 : (hh + 1) * 512],
                                lhsT=kT_sb[par : par + 64, 0, hp,
                                           si * 128 : (si + 1) * 128],
                                rhs=qT_sb[par : par + 64, 0, hp, t0 : t0 + 512],
                                start=True,
                                stop=True,
                            )
                    if si == TUNE["flush_si"] and flush_prev is not None:
                        flush_prev()
                    ready = [e for e in pend if e[2] <= si]
                    for e in ready:
                        pend.remove(e)
                        emit_pv(e[0], e[1])
                    for w in works_at.get(si, ()):
                        w()
                    pe = flow.tile([128, 1024], DT, tag="pe")
                    nc.scalar.activation(pe[:], ps[:], Act.Exp, scale=SCALING)
                    pw = flow.tile([128, 1024], DT, tag="pw")
                    on_pool = si in pool_si and si < SC - 2
                    eng = nc.gpsimd if on_pool else nc.vector
                    if si >= SC - 2:
                        # per-head halves: the flush PVs of head 0 start as
                        # soon as its half of pw is ready
                        for hh in range(2):
                            eng.tensor_mul(
                                pw[:, hh * 512 : (hh + 1) * 512],
                                pe[:, hh * 512 : (hh + 1) * 512],
                                ebm_sb[:, si, jb, :],
                            )
                    else:
                        eng.tensor_mul(
                            pw.rearrange("p (c t) -> p c t", c=2),
                            pe.rearrange("p (c t) -> p c t", c=2),
                            ebm_sb[:, si, jb, :]
                            .rearrange("p (c t) -> p c t", c=1)
                            .broadcast_to([128, 2, 512]),
                        )
                    # the last two steps' PV stay pending into flush, so the
                    # next block's first scores matmul is never queued behind
                    # this block's tail PVs on the in-order PE.
                    pend.append((si, pw, si + max(
                        pv_defer,
                        TUNE["defer_pool"] if on_pool
                        else (2 if si >= SC - 2 else 1))))

                pvbf = pvbfp.tile([128, 4, 2, D], DT, tag="pvbf", name=f"pvbf{jb}_{hp}")

                def flush():
                    pend.sort(key=lambda e: e[0])
                    for psi, ppw, _ in pend:
                        emit_pv(psi, ppw)
                    pend.clear()
                    for hh in range(2):
                        rec = small.tile([128, 4], F32, tag="rec")
                        nc.vector.reciprocal(rec[:], pvu[hh][:, :, D])
                        nc.vector.tensor_mul(
                            pvbf[:, :, hh, :],
                            pvu[hh][:, :, 0:D],
                            rec.rearrange("p (c o) -> p c o", o=1).broadcast_to([128, 4, D]),
                        )
                return pvbf, flush

            def tp_work(th, j, hp, pvbf, pvt, aux_ps):
                # transpose [t,jd] -> [jd,t] for one head pair via the DMA
                # xbar (keeps PE/DVE free; rides the idle SP DMA queue)
                if TUNE["dma_tp"]:
                    for tcc in range(4):
                        nc.sync.dma_start_transpose(
                            out=pvt[:, hp, j * 512 + tcc * 128
                                    : j * 512 + (tcc + 1) * 128],
                            in_=pvbf[:, tcc, :, :],
                        )
                    return
                tp = aux_ps.tile(
                    [128, 4, 128], DT, tag="aux", name=f"tp{th}_{j}_{hp}"
                )
                for tcc in range(4):
                    nc.tensor.transpose(
                        tp[:, tcc, :], pvbf[:, tcc, :, :], ident[:]
                    )
                nc.vector.tensor_copy(
                    pvt[:, hp, j * 512 : (j + 1) * 512],
                    tp.rearrange("p a b -> p (a b)"),
                )

            def ti_work(th, j, ti, pvt, po_pools, tail=False):
                # out-projection for one 128-row t-chunk
                trow = th * 1024 + j * 512 + ti * 128
                ot = otp.tile([128, E], F16, tag="ot")
                for ei in range(2):
                    pool, tag = po_pools[(ti * 2 + ei) % len(po_pools)]
                    po = pool.tile([128, 512], F32, tag=tag, name=f"po{trow}_{ei}")
                    for mc in range(MC):
                        nc.tensor.matmul(
                            po[:],
                            lhsT=pvt[:, mc, j * 512 + ti * 128 : j * 512 + (ti + 1) * 128],
                            rhs=wo_sb[:, mc, ei * 512 : (ei + 1) * 512],
                            start=(mc == 0),
                            stop=(mc == MC - 1),
                        )
                    if tail and TUNE["tail_split"]:
                        # latency-critical: halve each copy across both engines
                        # and DMA each 512-col piece as soon as it's ready
                        nc.vector.tensor_copy(
                            ot[:, ei * 512 : ei * 512 + 256], po[:, 0:256])
                        nc.scalar.copy(
                            ot[:, ei * 512 + 256 : (ei + 1) * 512], po[:, 256:512])
                        nc.sync.dma_start(
                            out=out_p[trow : trow + 128, ei * 512 : (ei + 1) * 512],
                            in_=ot[:, ei * 512 : (ei + 1) * 512])
                    elif tail and ei == 1:
                        nc.scalar.copy(ot[:, 512:1024], po[:])
                    else:
                        nc.vector.tensor_copy(
                            ot[:, ei * 512 : (ei + 1) * 512], po[:])
                if not (tail and TUNE["tail_split"]):
                    nc.sync.dma_start(out=out_p[trow : trow + 128, :], in_=ot[:])

            # ---- emission schedule ----
            # Block order: (hp0,j0), (hp1,j0), (hp0,j1), (hp1,j1) per t-half,
            # so block 2 needs no new DMA data.  drain(th,j) becomes ready
            # after the second block of its (th,j) pair and interleaves into
            # the next block.
            def wl(*fs):
                return list(fs)

            with tc.tile_pool(name="projps", bufs=2, space="PSUM") as proj_ps:
                # keep-alive matmuls paced by the DVE delay chain
                for i in range(3):
                    ka = proj_ps.tile([128, 16], F32, tag="proj", name=f"ka{i}")
                    nc.tensor.matmul(
                        ka[:],
                        lhsT=delay_t[i][0:1, 0:128],
                        rhs=delay_t[i][0:1, 0:16],
                        start=True,
                        stop=True,
                    )

                k_work(0, 0, proj_ps, ostep=256, act=True)
                q_work(0, 0, proj_ps, act=True)
                # remaining input DMAs (queue order continues)
                dma_x(xq_t[2], xqr, 1024, 1536)
                dma_ebm(2, 0)
                dma_ebm(2, 1)
                dma_x(xq_t[3], xqr, 1536, 2048)
                dma_ebm(3, 0)
                dma_ebm(3, 1)
                nc.sync.dma_start(
                    out=wo_sb[:], in_=woT.rearrange("(c p) e -> p c e", p=128)
                )

                w1 = {}
                for i in range(SC):
                    w1.setdefault(min(i + 1, SC - 1), []).append(
                        lambda i=i: v_work(i, 0, proj_ps))
                w1.setdefault(max(1, SCH - 1), []).append(
                    lambda: k_work(1, 0, proj_ps))
                w1.setdefault(max(1, SC - 2), []).append(
                    lambda: k_work(0, 1, proj_ps))
                w1.setdefault(max(1, SC - 3), []).append(
                    lambda: q_work(1, 0, proj_ps))
                b1, f1 = attn_block(0, 0, 0, pv_defer=2, works_at=w1)
                w2 = {}
                for i in range(SC):
                    w2.setdefault(min(i + 1, SC - 1), []).append(
                        lambda i=i: v_work(i, 1, proj_ps))
                w2.setdefault(min(2, SC - 1), []).append(
                    lambda: k_work(1, 1, proj_ps))
                w2.setdefault(SC - 2, []).append(
                    lambda: q_work(0, 1, proj_ps))
                w2.setdefault(SC - 1, []).append(
                    lambda: q_work(1, 1, proj_ps))
                b2, f2 = attn_block(0, 1, 0, flush_prev=f1, pv_defer=2,
                                    works_at=w2)

            with tc.tile_pool(name="auxps", bufs=2, space="PSUM") as aux_ps:
                app = [(aux_ps, "aux")]

                def tpw(th, j, hp, pvbf, pvt):
                    return lambda: tp_work(th, j, hp, pvbf, pvt, aux_ps)

                def tiw(th, j, ti, pvt):
                    return lambda: ti_work(th, j, ti, pvt, app)

                # each drain (2 transposes + 4 out-proj chunks) is spread
                # over the two following blocks to keep DVE under budget
                pvt0 = pvtp.tile([128, MC, 1024], DT, tag="pvt", name="pvt0")
                b3, f3 = attn_block(
                    0, 0, 1, flush_prev=f2, pool_si=TUNE["pool_si_drain"],
                    works_at={
                        1: wl(tpw(0, 0, 0, b1, pvt0)),
                        2: wl(tpw(0, 0, 1, b2, pvt0)),
                        4: wl(tiw(0, 0, 0, pvt0)),
                        6: wl(tiw(0, 0, 1, pvt0)),
                    },
                )
                b4, f4 = attn_block(
                    0, 1, 1, flush_prev=f3,
                    works_at={
                        1: wl(tiw(0, 0, 2, pvt0)),
                        2: wl(lambda: q_work(0, 2, aux_ps, tag="aux")),
                        3: wl(tiw(0, 0, 3, pvt0)),
                        5: wl(lambda: q_work(1, 2, aux_ps, tag="aux")),
                    },
                )
                b5, f5 = attn_block(
                    1, 0, 0, flush_prev=f4, pool_si=TUNE["pool_si_drain"],
                    works_at={
                        1: wl(tpw(0, 1, 0, b3, pvt0)),
                        2: wl(tpw(0, 1, 1, b4, pvt0)),
                        4: wl(tiw(0, 1, 0, pvt0)),
                        6: wl(tiw(0, 1, 1, pvt0)),
                    },
                )
                b6, f6 = attn_block(
                    1, 1, 0, flush_prev=f5,
                    works_at={
                        1: wl(tiw(0, 1, 2, pvt0)),
                        2: wl(lambda: q_work(0, 3, aux_ps, tag="aux")),
                        3: wl(tiw(0, 1, 3, pvt0)),
                        5: wl(lambda: q_work(1, 3, aux_ps, tag="aux")),
                    },
                )
                pvt1 = pvtp.tile([128, MC, 1024], DT, tag="pvt", name="pvt1")
                b7, f7 = attn_block(
                    1, 0, 1, flush_prev=f6, pool_si=TUNE["pool_si_drain"],
                    works_at={
                        1: wl(tpw(1, 0, 0, b5, pvt1)),
                        2: wl(tpw(1, 0, 1, b6, pvt1)),
                        4: wl(tiw(1, 0, 0, pvt1)),
                        6: wl(tiw(1, 0, 1, pvt1)),
                    },
                )
                # b7's transpose can run inside b8 (b7 flushed at b8.si0);
                # only b8's transpose + the last out-projections remain as tail.
                b8, f8 = attn_block(
                    1, 1, 1, flush_prev=f7,
                    works_at={
                        1: wl(tiw(1, 0, 2, pvt1)),
                        3: wl(tiw(1, 0, 3, pvt1)),
                        5: wl(tpw(1, 1, 0, b7, pvt1)),
                    },
                )
                f8()
                tp_work(1, 1, 1, b8, pvt1, aux_ps)
                tail_pp = [(aux_ps, "aux"), (aux_ps, "aux"),
                           (sc_ps, "sc"), (sc_ps, "sc")]
                for ti in range(4):
                    ti_work(1, 1, ti, pvt1, tail_pp, tail=True)

    nc.compile()
    return nc


_NC_CACHE = {}
_NC = None
_LAST_RES = None


def _get_nc(SP=S):
    global _NC
    if SP not in _NC_CACHE:
        _NC_CACHE[SP] = _build_nc(SP)
    _NC = _NC_CACHE[SP]
    return _NC


def kernel(query, key, value, attn_bias, key_padding_mask,
           in_proj_w, in_proj_b, out_proj_w, out_proj_b):
    from concourse.bass_utils import run_bass_kernel_spmd

    query = np.asarray(query, np.float32)
    key = np.asarray(key, np.float32)
    value = np.asarray(value, np.float32)
    attn_bias = np.asarray(attn_bias, np.float32)
    key_padding_mask = np.asarray(key_padding_mask, bool)
    in_proj_w = np.asarray(in_proj_w, np.float32)
    in_proj_b = np.asarray(in_proj_b, np.float32)
    out_proj_w = np.asarray(out_proj_w, np.float32)
    out_proj_b = np.asarray(out_proj_b, np.float32)

    w_q, w_k, w_v = in_proj_w[:E], in_proj_w[E : 2 * E], in_proj_w[2 * E :]
    b_q, b_k, b_v = in_proj_b[:E], in_proj_b[E : 2 * E], in_proj_b[2 * E :]

    ebm_base = np.exp(attn_bias[0]).T  # [S, T]

    # compact away masked keys (their softmax weight is exactly 0);
    # pad the kept set to a multiple of 128 with ebm == 0 rows.
    idx = {b: np.nonzero(~key_padding_mask[b])[0] for b in range(B)}
    s_eff = max(len(idx[b]) for b in range(B))
    SP = max(128, -(-s_eff // 128) * 128)

    xT = {}
    for b in range(B):
        n = len(idx[b])
        kc = np.zeros((SP, E), np.float32)
        kc[:n] = key[b][idx[b]]
        vc = np.zeros((SP, E), np.float32)
        vc[:n] = value[b][idx[b]]
        ec = np.zeros((SP, T), np.float32)
        ec[:n] = ebm_base[idx[b]]
        # j-blocked ebm layout: [T/512, SP, 512]
        e4 = np.ascontiguousarray(
            ec.reshape(SP, T // 512, 512).transpose(1, 0, 2)
        ).astype(BF16)
        xT[b] = (
            query[b].T.astype(BF16),
            kc.T.astype(BF16),
            vc.T.astype(BF16),
            e4,
        )

    in_maps = []
    for core in range(NCORES):
        b, hg = core // 4, core % 4
        rows = slice(hg * JD, (hg + 1) * JD)
        qT, kT, vT, e4 = xT[b]
        in_maps.append({
            "xqT": qT, "xkT": kT, "xvT": vT, "ebm4": e4,
            "wqT": np.ascontiguousarray(w_q[rows].T).astype(BF16),
            "wkT": np.ascontiguousarray(w_k[rows].T).astype(BF16),
            "wvT": np.ascontiguousarray(w_v[rows].T).astype(BF16),
            "woT": np.ascontiguousarray(out_proj_w[:, rows].T).astype(BF16),
            "bqk": np.ascontiguousarray(np.concatenate([
                b_q[rows].reshape(JD // 128, 128).T,
                b_k[rows].reshape(JD // 128, 128).T], axis=1)),
        })

    nc = _get_nc(SP)
    import os
    trace = os.environ.get("KERNEL_TRACE", "") == "1"
    kwargs = {}
    if trace:
        kwargs["tmpdir"] = os.environ.get("KERNEL_TRACE_DIR") or None
    res = run_bass_kernel_spmd(
        nc, in_maps, core_ids=list(range(NCORES)), trace=trace, **kwargs
    )
    global _LAST_RES
    _LAST_RES = res

    out = np.zeros((B, T, E), np.float32)
    for core in range(NCORES):
        out[core // 4] += res.results[core]["out_p"].astype(np.float32)
    out += (out_proj_b + out_proj_w @ b_v)[None, None, :]
    return out
